# revision 19
# baseline (speedup 1.0000x reference)
"""Trainium2 Bass kernel for nn_LASCC (sparse patch-correlation attention + top-k).

Math (per batch element b):
  x_hat = L2-normalize(x, dim=channels)
  z_p[c, n] = x_hat at the two in-patch diagonal pixels (p=0: (0,0), p=1: (1,1))
  C_p = z_p^T z_p                  (1024x1024 normalized correlation, symmetric)
  C_2 = (C_0 + C_1)/2              (avg map)
  s_q = alpha * mask * C_q
  A_q = softmax_row(s) * softmax_col(s) = exp(2 a_q s)*u[n]*u[m], u = 1/rowsum(exp(a_q s))
  out pixel with patch n, map q: top-3 over m of A_q[n, m]

Log-domain top-k: order over m of A[n, m] == order of T[n, m] = s[n, m]
+ ln(u_m)/(2 a_q), so the F-phase is ONE fp16 2x tensor-add + max8; the
top-3 VALUES are recovered with a tiny exp on [128, 8, 3]:
out = exp(2 a_q * T_top3) * u_n.  Only one full-size exp per chunk
remains (for the row sums), and all ACT functions used (Exp, Ln,
Square, Copy) live in the single natural_log_exp_and_others table --
zero ACT table reloads in steady state.

Engine split per (b, q) stage (8 chunks):
  PE   : Gram matmuls (fp16 z, 1 cyc/row)
  DVE  : mask-mult from PSUM for 5 chunks + T-add (fp16 2x) + max8
  ACT  : G->SBUF copies for 3 chunks + exp (accum rowsums) + tiny Ln/exp
  Pool : mask-mult (SBUF) for the 3 ACT-copied chunks
  DMA  : ln(u)/2a roundtrip + stride-0 partition broadcast
"""
import numpy as np

import concourse.bass as bass
import concourse.mybir as mybir
from concourse import bacc
from concourse.tile import TileContext
from concourse.bass_utils import run_bass_kernel_spmd

F32 = mybir.dt.float32
FP16 = mybir.dt.float16
AF = mybir.ActivationFunctionType
ALU = mybir.AluOpType

B_FULL = 16
N_CORES = 8
B_LOC = B_FULL // N_CORES  # 2
C = 128
H = W = 64
NPH = 32
NP = 1024  # patches
PS = 2
TOPK = 3
NCHUNK = NP // 128  # 8
N_DVE_MASK = 5  # chunks whose mask-mult runs on DVE (rest: ACT copy + Pool)

LAST_EXEC_NS = None


def _free_bcast_ap(tile_ap, free_dims):
    """Manual AP: same tensor/offset, free dims replaced (list of [stride, n])."""
    ap = tile_ap
    new = [ap.ap[0]] + [list(d) for d in free_dims]
    return bass.AP(ap.tensor, ap.offset, new)


def build_nc():
    nc = bacc.Bacc(trn_type="TRN2")

    x_d = nc.dram_tensor("x", [B_LOC, C, H * W], F32, kind="ExternalInput")
    alpha_d = nc.dram_tensor("alpha", [128, 1], F32, kind="ExternalInput")
    mask_d = nc.dram_tensor("mask", [NP, NP], FP16, kind="ExternalInput")
    out_d = nc.dram_tensor("out", [B_LOC, 3, NP, TOPK], F32, kind="ExternalOutput")

    with TileContext(nc) as tc:
        with tc.tile_pool(name="const", bufs=1) as cpool, \
             tc.tile_pool(name="zp", bufs=1) as zpool, \
             tc.tile_pool(name="slab", bufs=2) as slabp, \
             tc.tile_pool(name="ssl", bufs=2) as sslp, \
             tc.tile_pool(name="work", bufs=3) as work, \
             tc.tile_pool(name="wsc", bufs=3) as wscp, \
             tc.tile_pool(name="small", bufs=3) as small, \
             tc.tile_pool(name="ps", bufs=2, space="PSUM") as ps, \
             tc.tile_pool(name="psn", bufs=1, space="PSUM") as psn, \
             tc.tile_pool(name="dsc", bufs=3, space="DRAM") as dsc:

            # ---- constants
            ones_k = cpool.tile([128, 1], F32)    # colsum matmul lhsT
            nc.vector.memset(ones_k, 1.0)
            ones_r = cpool.tile([1, 128], F32)    # K=1 bcast matmul lhsT
            nc.vector.memset(ones_r, 1.0)
            av = cpool.tile([128, 1], F32)        # alpha
            nc.sync.dma_start(av, alpha_d[:, :])
            av_h = cpool.tile([128, 1], F32)      # alpha/2
            nc.vector.tensor_scalar_mul(av_h, av, 0.5)
            av_d = cpool.tile([128, 1], F32)      # 2*alpha
            nc.vector.tensor_scalar_mul(av_d, av, 2.0)
            rav2 = cpool.tile([128, 1], F32)      # 1/(2*alpha)
            nc.vector.reciprocal(rav2, av_d)
            rav1 = cpool.tile([128, 1], F32)      # 1/alpha
            nc.vector.reciprocal(rav1, av)
            scale_E = [av, av, av_h]     # exp scale a_q for rowsums
            scale_T = [av_d, av_d, av]   # 2*a_q for the tiny value exp
            scale_L = [rav2, rav2, rav1]  # 1/(2*a_q) for ln(u)

            # ---- mask (fp16, [p, i, m] chunk layout), from host
            mask_sb = cpool.tile([128, NCHUNK, NP], FP16)
            nc.sync.dma_start(
                mask_sb, mask_d[:, :].rearrange("(i p) m -> p i m", p=128))

            # ---- phase N: per-pixel channel norms + normalized z (fp16)
            chains = []
            for b in range(B_LOC):
                xs = slabp.tile([128, H * W], F32, name=f"xs{b}", tag="xs")
                nc.sync.dma_start(xs, x_d[b])
                xr = xs.rearrange("c (i r j s) -> c r s i j", r=PS, s=PS, j=NPH)
                for p in range(PS):
                    chains.append((b, p, xr[:, p, p]))

            inv = {}
            for b, p, zv in chains:  # inv = exp(-0.5 ln nrm2) = 1/sqrt(nrm2)
                zsq = work.tile([128, NP], F32, name="zsq", tag="zsq", bufs=2)
                nc.scalar.activation(
                    zsq.rearrange("c (a b) -> c a b", a=NPH), zv, AF.Square)
                nrm = psn.tile([1, NP], F32, name="nrm", tag="nrm")
                for h in range(2):
                    nc.tensor.matmul(nrm[:, 512 * h:512 * (h + 1)], ones_k,
                                     zsq[:, 512 * h:512 * (h + 1)],
                                     start=True, stop=True)
                lnn = small.tile([1, NP], F32, name="lnn", tag="lnn", bufs=2)
                nc.scalar.activation(lnn, nrm, AF.Ln)
                inv1 = small.tile([1, NP], F32, name="inv1", tag="inv1", bufs=2)
                nc.scalar.activation(inv1, lnn, AF.Exp, scale=-0.5)
                inv[(b, p)] = inv1
            zp = {}
            for b, p, zv in chains:
                ibc = psn.tile([128, NP], F32, name="ibc", tag="ibc")
                for h in range(2):
                    nc.tensor.matmul(ibc[:, 512 * h:512 * (h + 1)], ones_r,
                                     inv[(b, p)][:, 512 * h:512 * (h + 1)],
                                     start=True, stop=True)
                z = zpool.tile([128, NP], FP16, name=f"z{b}{p}", tag=f"z{b}{p}",
                               bufs=1)
                nc.vector.tensor_tensor(
                    out=z.rearrange("c (a b) -> c a b", a=NPH), in0=zv,
                    in1=ibc.rearrange("c (a b) -> c a b", a=NPH), op=ALU.mult)
                zp[(b, p)] = z

            # ---- phase M: six (b, q) stages, software-pipelined
            def emit_E(b, q):
                s_sl = sslp.tile([128, NCHUNK, NP], FP16, name="s_sl",
                                 tag="s_sl")
                rT = small.tile([128, NCHUNK], F32, name="rT", tag="rT")
                srcs = [zp[(b, 0)]] if q == 0 else \
                       [zp[(b, 1)]] if q == 1 else [zp[(b, 0)], zp[(b, 1)]]
                for i in range(NCHUNK):
                    G = ps.tile([128, NP], F32, name="G", tag="G")
                    for h in range(2):
                        for si, zs in enumerate(srcs):
                            nc.tensor.matmul(
                                G[:, 512 * h:512 * (h + 1)],
                                zs[:, 128 * i:128 * (i + 1)],
                                zs[:, 512 * h:512 * (h + 1)],
                                start=(si == 0), stop=(si == len(srcs) - 1))
                    if i < N_DVE_MASK:
                        nc.vector.tensor_tensor(
                            out=s_sl[:, i, :], in0=G, in1=mask_sb[:, i, :],
                            op=ALU.mult)
                    else:
                        Gs = work.tile([128, NP], FP16, name="Gs", tag="Gs",
                                       bufs=3)
                        nc.scalar.copy(Gs, G)
                        nc.gpsimd.tensor_tensor(
                            out=s_sl[:, i, :], in0=Gs, in1=mask_sb[:, i, :],
                            op=ALU.mult)
                    e_scr = work.tile([128, NP], FP16, name="e_scr",
                                      tag="e_scr", bufs=2)
                    nc.scalar.activation(e_scr, s_sl[:, i, :], AF.Exp,
                                         scale=scale_E[q],
                                         accum_out=rT[:, i:i + 1])
                u8 = small.tile([128, NCHUNK], F32, name="u8", tag="u8")
                nc.vector.reciprocal(u8, rT)
                lnu = small.tile([128, NCHUNK], F32, name="lnu", tag="lnu")
                nc.scalar.activation(lnu, u8, AF.Ln)
                lnsc = small.tile([128, NCHUNK], FP16, name="lnsc", tag="lnsc")
                nc.vector.tensor_scalar_mul(lnsc, lnu, scale_L[q][:, 0:1])
                l_dram = dsc.tile([NP], FP16, name="l_dram", tag="l_dram")
                nc.sync.dma_start(
                    l_dram[:].rearrange("(i p) -> p i", p=128), lnsc)
                lnubc = wscp.tile([128, NP], FP16, name="lnubc", tag="lnubc",
                                  bufs=2)
                src = bass.AP(l_dram[:].tensor, l_dram[:].offset,
                              [[0, 128], [1, NP]])
                nc.sync.dma_start(lnubc, src)
                return dict(s_sl=s_sl, lnubc=lnubc, u8=u8, b=b, q=q)

            def emit_F(stg):
                b, q = stg["b"], stg["q"]
                t8 = small.tile([128, NCHUNK, 8], FP16, name="t8", tag="t8")
                for i in range(NCHUNK):
                    T_sc = wscp.tile([128, NP], FP16, name="T_sc", tag="T_sc",
                                     bufs=3)
                    nc.vector.tensor_tensor(out=T_sc, in0=stg["s_sl"][:, i, :],
                                            in1=stg["lnubc"], op=ALU.add)
                    nc.vector.max(out=t8[:, i, :], in_=T_sc)
                # values: out = exp(2 a_q T) * u_n
                tex = small.tile([128, NCHUNK, TOPK], F32, name="tex",
                                 tag="tex")
                nc.scalar.activation(tex, t8[:, :, :TOPK], AF.Exp,
                                     scale=scale_T[q])
                oacc = small.tile([128, NCHUNK, TOPK], F32, name="oacc",
                                  tag="oacc")
                u8b = _free_bcast_ap(stg["u8"][:, :], [[1, NCHUNK], [0, TOPK]])
                nc.vector.tensor_tensor(out=oacc, in0=tex, in1=u8b,
                                        op=ALU.mult)
                dst = out_d[b, q].rearrange("(i p) k -> p i k", p=128)
                nc.sync.dma_start(dst, oacc)

            stages = [(b, q) for b in range(B_LOC) for q in range(3)]
            pending = None
            for (b, q) in stages:
                stg = emit_E(b, q)
                if pending is not None:
                    emit_F(pending)
                pending = stg
            emit_F(pending)

    nc.compile()
    return nc


_NC_CACHE = None


def _get_nc():
    global _NC_CACHE
    if _NC_CACHE is None:
        _NC_CACHE = build_nc()
    return _NC_CACHE


def _build_mask() -> np.ndarray:
    """(1 - gaussian) self-suppression mask, [NP, NP] fp16."""
    rat_s = np.float32(0.05)
    sr = np.float32(NPH) * rat_s
    ind_r = np.arange(NPH, dtype=np.float32).reshape(1, NPH, 1)
    ind_c = np.arange(NPH, dtype=np.float32).reshape(1, 1, NPH)
    cent = np.arange(NPH, dtype=np.float32)
    cent_r = np.repeat(cent, NPH).reshape(NP, 1, 1)
    cent_c = np.tile(cent, NPH).reshape(NP, 1, 1)
    g = np.exp(-((ind_r - cent_r) ** 2) / (2.0 * sr * sr)) * np.exp(
        -((ind_c - cent_c) ** 2) / (2.0 * sr * sr)
    )
    return (1.0 - g).reshape(NP, NP).astype(np.float16)


def kernel(x: np.ndarray, alpha: np.ndarray) -> np.ndarray:
    global LAST_EXEC_NS
    x = np.ascontiguousarray(np.asarray(x, dtype=np.float32))
    alpha_arr = np.full((128, 1), np.float32(np.asarray(alpha)),
                        dtype=np.float32)
    mask = _build_mask()

    nc = _get_nc()
    in_maps = []
    for core in range(N_CORES):
        xs = x[core * B_LOC:(core + 1) * B_LOC].reshape(B_LOC, C, H * W)
        in_maps.append({"x": np.ascontiguousarray(xs), "alpha": alpha_arr,
                        "mask": mask})
    res = run_bass_kernel_spmd(nc, in_maps, core_ids=list(range(N_CORES)))
    LAST_EXEC_NS = res.exec_time_ns

    # assemble: out[bg, k, 2i+dr, 2j+dc] from T_q[b, n=i*32+j, k]
    out = np.empty((B_FULL, TOPK, H, W), dtype=np.float32)
    for core in range(N_CORES):
        t = res.results[core]["out"]  # [B_LOC, 3, NP, TOPK]
        for bl in range(B_LOC):
            bg = core * B_LOC + bl
            tq = t[bl].reshape(3, NPH, NPH, TOPK).transpose(0, 3, 1, 2)
            out[bg, :, 0::2, 0::2] = tq[0]
            out[bg, :, 1::2, 1::2] = tq[1]
            out[bg, :, 0::2, 1::2] = tq[2]
            out[bg, :, 1::2, 0::2] = tq[2]
    return out


# revision 20
# speedup vs baseline: 1.1480x; 1.1480x over previous
"""Trainium2 Bass kernel for nn_LASCC (sparse patch-correlation attention + top-k).

Math (per batch element b):
  x_hat = L2-normalize(x, dim=channels)
  z_p[c, n] = x_hat at the two in-patch diagonal pixels (p=0: (0,0), p=1: (1,1))
  C_p = z_p^T z_p                  (1024x1024 normalized correlation, symmetric)
  C_2 = (C_0 + C_1)/2              (avg map)
  s_q = alpha * mask * C_q
  A_q = exp(2 a_q t_q) * u[n] * u[m],  t_q = mask*C_q-ish slab, u = 1/rowsum(exp(a_q t_q))
  out pixel with patch n, map q: top-3 over m of A_q[n, m]

Slabs store t_q: t_0 = mask*C_0, t_1 = mask*C_1, t_2 = t_0 + t_1 (so
q=2 needs NO matmuls and no mask pass: a_2 = alpha/2 instead of alpha).

Log-domain top-k: order over m of A[n, m] == order of T[n, m] = t[n, m]
+ ln(u_m)/a2_q (a2_q = 2 a_q), so the F-phase is ONE fp16 2x tensor-add
+ max8; the top-3 VALUES are recovered with a tiny exp on [128, 8, 3]:
out = exp(a2_q * T_top3) * u_n.  One full-size exp per chunk remains
(row sums).  ACT functions (Exp, Ln, Square, Copy) mostly share the
natural_log_exp_and_others table.

E(k+1) and F(k) are interleaved per chunk at emission so each engine's
in-order stream alternates ready work instead of head-of-line blocking.
"""
import numpy as np

import concourse.bass as bass
import concourse.mybir as mybir
from concourse import bacc
from concourse.tile import TileContext
from concourse.bass_utils import run_bass_kernel_spmd

F32 = mybir.dt.float32
FP16 = mybir.dt.float16
AF = mybir.ActivationFunctionType
ALU = mybir.AluOpType

B_FULL = 16
N_CORES = 8
B_LOC = B_FULL // N_CORES  # 2
C = 128
H = W = 64
NPH = 32
NP = 1024
PS = 2
TOPK = 3
NCHUNK = NP // 128  # 8

N_DVE_MASK = 3   # q0/q1 chunks whose mask-mult runs on DVE (rest ACT+Pool)
N_DVE_TADD = 6   # F-phase T-adds on DVE (rest Pool)
N_DVE_S2 = 4     # q2 slab-adds on DVE (rest Pool)

LAST_EXEC_NS = None


def _free_bcast_ap(tile_ap, free_dims):
    ap = tile_ap
    new = [ap.ap[0]] + [list(d) for d in free_dims]
    return bass.AP(ap.tensor, ap.offset, new)


def build_nc():
    nc = bacc.Bacc(trn_type="TRN2")

    x_d = nc.dram_tensor("x", [B_LOC, C, H * W], F32, kind="ExternalInput")
    alpha_d = nc.dram_tensor("alpha", [128, 1], F32, kind="ExternalInput")
    mask_d = nc.dram_tensor("mask", [NP, NP], FP16, kind="ExternalInput")
    out_d = nc.dram_tensor("out", [B_LOC, 3, NP, TOPK], F32, kind="ExternalOutput")

    with TileContext(nc) as tc:
        with tc.tile_pool(name="const", bufs=1) as cpool, \
             tc.tile_pool(name="zp", bufs=1) as zpool, \
             tc.tile_pool(name="slab", bufs=2) as slabp, \
             tc.tile_pool(name="ssl", bufs=4) as sslp, \
             tc.tile_pool(name="work", bufs=3) as work, \
             tc.tile_pool(name="wsc", bufs=3) as wscp, \
             tc.tile_pool(name="small", bufs=3) as small, \
             tc.tile_pool(name="ps", bufs=2, space="PSUM") as ps, \
             tc.tile_pool(name="psn", bufs=1, space="PSUM") as psn, \
             tc.tile_pool(name="dsc", bufs=3, space="DRAM") as dsc:

            # ---- constants
            ones_k = cpool.tile([128, 1], FP16)   # colsum matmul lhsT
            nc.vector.memset(ones_k, 1.0)
            ones_r = cpool.tile([1, 128], FP16)   # K=1 bcast matmul lhsT
            nc.vector.memset(ones_r, 1.0)
            av = cpool.tile([128, 1], F32)        # alpha
            nc.sync.dma_start(av, alpha_d[:, :])
            av_h = cpool.tile([128, 1], F32)      # alpha/2
            nc.vector.tensor_scalar_mul(av_h, av, 0.5)
            av_d = cpool.tile([128, 1], F32)      # 2*alpha
            nc.vector.tensor_scalar_mul(av_d, av, 2.0)
            rav2 = cpool.tile([128, 1], F32)      # 1/(2*alpha)
            nc.vector.reciprocal(rav2, av_d)
            rav1 = cpool.tile([128, 1], F32)      # 1/alpha
            nc.vector.reciprocal(rav1, av)
            scale_E = [av, av, av_h]      # a_q for the rowsum exp
            scale_T = [av_d, av_d, av]    # 2 a_q for the tiny value exp
            scale_L = [rav2, rav2, rav1]  # 1/(2 a_q) for ln(u)

            # ---- mask (fp16, [p, i, m] chunk layout)
            mask_sb = cpool.tile([128, NCHUNK, NP], FP16)
            nc.sync.dma_start(
                mask_sb, mask_d[:, :].rearrange("(i p) m -> p i m", p=128))

            # ---- phase N
            chains = []
            for b in range(B_LOC):
                xs = slabp.tile([128, H * W], F32, name=f"xs{b}", tag="xs")
                nc.sync.dma_start(xs, x_d[b])
                xr = xs.rearrange("c (i r j s) -> c r s i j", r=PS, s=PS, j=NPH)
                for p in range(PS):
                    chains.append((b, p, xr[:, p, p]))

            inv = {}
            for b, p, zv in chains:  # inv = exp(-0.5 ln nrm2)
                zsq = work.tile([128, NP], FP16, name="zsq", tag="zsq", bufs=2)
                nc.scalar.activation(
                    zsq.rearrange("c (a b) -> c a b", a=NPH), zv, AF.Square)
                nrm = psn.tile([1, NP], F32, name="nrm", tag="nrm")
                for h in range(2):
                    nc.tensor.matmul(nrm[:, 512 * h:512 * (h + 1)], ones_k,
                                     zsq[:, 512 * h:512 * (h + 1)],
                                     start=True, stop=True)
                lnn = small.tile([1, NP], F32, name="lnn", tag="lnn", bufs=2)
                nc.scalar.activation(lnn, nrm, AF.Ln)
                inv1 = small.tile([1, NP], FP16, name="inv1", tag="inv1",
                                  bufs=2)
                nc.scalar.activation(inv1, lnn, AF.Exp, scale=-0.5)
                inv[(b, p)] = inv1
            zp = {}
            for b, p, zv in chains:
                ibc = psn.tile([128, NP], F32, name="ibc", tag="ibc")
                for h in range(2):
                    nc.tensor.matmul(ibc[:, 512 * h:512 * (h + 1)], ones_r,
                                     inv[(b, p)][:, 512 * h:512 * (h + 1)],
                                     start=True, stop=True)
                z = zpool.tile([128, NP], FP16, name=f"z{b}{p}", tag=f"z{b}{p}",
                               bufs=1)
                nc.vector.tensor_tensor(
                    out=z.rearrange("c (a b) -> c a b", a=NPH), in0=zv,
                    in1=ibc.rearrange("c (a b) -> c a b", a=NPH), op=ALU.mult)
                zp[(b, p)] = z

            # ---- phase M: 6 stages, per-chunk interleaved software pipeline
            s_of = {}  # (b, q) -> slab

            def E_chunk(b, q, i, s_sl, rT):
                if q < 2:
                    zs = zp[(b, q)]
                    G = ps.tile([128, NP], F32, name="G", tag="G")
                    for h in range(2):
                        nc.tensor.matmul(
                            G[:, 512 * h:512 * (h + 1)],
                            zs[:, 128 * i:128 * (i + 1)],
                            zs[:, 512 * h:512 * (h + 1)],
                            start=True, stop=True)
                    if i < N_DVE_MASK:
                        nc.vector.tensor_tensor(
                            out=s_sl[:, i, :], in0=G, in1=mask_sb[:, i, :],
                            op=ALU.mult)
                    else:
                        Gs = work.tile([128, NP], FP16, name="Gs", tag="Gs",
                                       bufs=3)
                        nc.scalar.copy(Gs, G)
                        nc.gpsimd.tensor_tensor(
                            out=s_sl[:, i, :], in0=Gs, in1=mask_sb[:, i, :],
                            op=ALU.mult)
                else:
                    s0, s1 = s_of[(b, 0)], s_of[(b, 1)]
                    eng = nc.vector if i < N_DVE_S2 else nc.gpsimd
                    eng.tensor_tensor(out=s_sl[:, i, :], in0=s0[:, i, :],
                                      in1=s1[:, i, :], op=ALU.add)
                e_scr = work.tile([128, NP], FP16, name="e_scr", tag="e_scr",
                                  bufs=2)
                nc.scalar.activation(e_scr, s_sl[:, i, :], AF.Exp,
                                     scale=scale_E[q],
                                     accum_out=rT[:, i:i + 1])

            def E_tail(b, q, s_sl, rT):
                u8 = small.tile([128, NCHUNK], F32, name="u8", tag="u8")
                nc.vector.reciprocal(u8, rT)
                lnu = small.tile([128, NCHUNK], F32, name="lnu", tag="lnu")
                nc.scalar.activation(lnu, u8, AF.Ln)
                lnsc = small.tile([128, NCHUNK], FP16, name="lnsc", tag="lnsc")
                nc.vector.tensor_scalar_mul(lnsc, lnu, scale_L[q][:, 0:1])
                l_dram = dsc.tile([NP], FP16, name="l_dram", tag="l_dram")
                nc.sync.dma_start(
                    l_dram[:].rearrange("(i p) -> p i", p=128), lnsc)
                lnubc = wscp.tile([128, NP], FP16, name="lnubc", tag="lnubc",
                                  bufs=2)
                src = bass.AP(l_dram[:].tensor, l_dram[:].offset,
                              [[0, 128], [1, NP]])
                nc.sync.dma_start(lnubc, src)
                return dict(s_sl=s_sl, lnubc=lnubc, u8=u8, b=b, q=q)

            def F_chunk(stg, i, t8):
                eng = nc.vector if i < N_DVE_TADD else nc.gpsimd
                T_sc = wscp.tile([128, NP], FP16, name="T_sc", tag="T_sc",
                                 bufs=3)
                eng.tensor_tensor(out=T_sc, in0=stg["s_sl"][:, i, :],
                                  in1=stg["lnubc"], op=ALU.add)
                nc.vector.max(out=t8[:, i, :], in_=T_sc)

            def F_tail(stg, t8):
                b, q = stg["b"], stg["q"]
                tex = small.tile([128, NCHUNK, TOPK], F32, name="tex",
                                 tag="tex")
                nc.scalar.activation(tex, t8[:, :, :TOPK], AF.Exp,
                                     scale=scale_T[q])
                oacc = small.tile([128, NCHUNK, TOPK], F32, name="oacc",
                                  tag="oacc")
                u8b = _free_bcast_ap(stg["u8"][:, :], [[1, NCHUNK], [0, TOPK]])
                nc.vector.tensor_tensor(out=oacc, in0=tex, in1=u8b,
                                        op=ALU.mult)
                dst = out_d[b, q].rearrange("(i p) k -> p i k", p=128)
                nc.sync.dma_start(dst, oacc)

            stages = [(b, q) for b in range(B_LOC) for q in range(3)]
            pending = None   # (stg, t8) awaiting F emission
            for (b, q) in stages:
                s_sl = sslp.tile([128, NCHUNK, NP], FP16, name="s_sl",
                                 tag="s_sl")
                s_of[(b, q)] = s_sl
                rT = small.tile([128, NCHUNK], F32, name="rT", tag="rT")
                if pending is not None:
                    pstg, pt8 = pending
                    for i in range(NCHUNK):
                        E_chunk(b, q, i, s_sl, rT)
                        F_chunk(pstg, i, pt8)
                    F_tail(pstg, pt8)
                else:
                    for i in range(NCHUNK):
                        E_chunk(b, q, i, s_sl, rT)
                stg = E_tail(b, q, s_sl, rT)
                t8 = small.tile([128, NCHUNK, 8], FP16, name="t8", tag="t8")
                pending = (stg, t8)
            pstg, pt8 = pending
            for i in range(NCHUNK):
                F_chunk(pstg, i, pt8)
            F_tail(pstg, pt8)

    nc.compile()
    return nc


_NC_CACHE = None


def _get_nc():
    global _NC_CACHE
    if _NC_CACHE is None:
        _NC_CACHE = build_nc()
    return _NC_CACHE


def _build_mask() -> np.ndarray:
    rat_s = np.float32(0.05)
    sr = np.float32(NPH) * rat_s
    ind_r = np.arange(NPH, dtype=np.float32).reshape(1, NPH, 1)
    ind_c = np.arange(NPH, dtype=np.float32).reshape(1, 1, NPH)
    cent = np.arange(NPH, dtype=np.float32)
    cent_r = np.repeat(cent, NPH).reshape(NP, 1, 1)
    cent_c = np.tile(cent, NPH).reshape(NP, 1, 1)
    g = np.exp(-((ind_r - cent_r) ** 2) / (2.0 * sr * sr)) * np.exp(
        -((ind_c - cent_c) ** 2) / (2.0 * sr * sr)
    )
    return (1.0 - g).reshape(NP, NP).astype(np.float16)


def kernel(x: np.ndarray, alpha: np.ndarray) -> np.ndarray:
    global LAST_EXEC_NS
    x = np.ascontiguousarray(np.asarray(x, dtype=np.float32))
    alpha_arr = np.full((128, 1), np.float32(np.asarray(alpha)),
                        dtype=np.float32)
    mask = _build_mask()

    nc = _get_nc()
    in_maps = []
    for core in range(N_CORES):
        xs = x[core * B_LOC:(core + 1) * B_LOC].reshape(B_LOC, C, H * W)
        in_maps.append({"x": np.ascontiguousarray(xs), "alpha": alpha_arr,
                        "mask": mask})
    res = run_bass_kernel_spmd(nc, in_maps, core_ids=list(range(N_CORES)))
    LAST_EXEC_NS = res.exec_time_ns

    out = np.empty((B_FULL, TOPK, H, W), dtype=np.float32)
    for core in range(N_CORES):
        t = res.results[core]["out"]
        for bl in range(B_LOC):
            bg = core * B_LOC + bl
            tq = t[bl].reshape(3, NPH, NPH, TOPK).transpose(0, 3, 1, 2)
            out[bg, :, 0::2, 0::2] = tq[0]
            out[bg, :, 1::2, 1::2] = tq[1]
            out[bg, :, 0::2, 1::2] = tq[2]
            out[bg, :, 1::2, 0::2] = tq[2]
    return out


# revision 21
# speedup vs baseline: 1.1823x; 1.0299x over previous
"""Trainium2 Bass kernel for nn_LASCC (sparse patch-correlation attention + top-k).

Math (per batch element b):
  x_hat = L2-normalize(x, dim=channels)
  z_p[c, n] = x_hat at the two in-patch diagonal pixels (p=0: (0,0), p=1: (1,1))
  C_p = z_p^T z_p                  (1024x1024 normalized correlation, symmetric)
  C_2 = (C_0 + C_1)/2              (avg map)
  s_q = alpha * mask * C_q
  A_q = exp(2 a_q t_q) * u[n] * u[m],  t_q = mask*C_q-ish slab, u = 1/rowsum(exp(a_q t_q))
  out pixel with patch n, map q: top-3 over m of A_q[n, m]

Slabs store t_q: t_0 = mask*C_0, t_1 = mask*C_1, t_2 = t_0 + t_1 (so
q=2 needs NO matmuls and no mask pass: a_2 = alpha/2 instead of alpha).

Log-domain top-k: order over m of A[n, m] == order of T[n, m] = t[n, m]
+ ln(u_m)/a2_q (a2_q = 2 a_q), so the F-phase is ONE fp16 2x tensor-add
+ max8; the top-3 VALUES are recovered with a tiny exp on [128, 8, 3]:
out = exp(a2_q * T_top3) * u_n.  One full-size exp per chunk remains
(row sums).  ACT functions (Exp, Ln, Square, Copy) mostly share the
natural_log_exp_and_others table.

E(k+1) and F(k) are interleaved per chunk at emission so each engine's
in-order stream alternates ready work instead of head-of-line blocking.
"""
import numpy as np

import concourse.bass as bass
import concourse.mybir as mybir
from concourse import bacc
from concourse.tile import TileContext
from concourse.bass_utils import run_bass_kernel_spmd

F32 = mybir.dt.float32
FP16 = mybir.dt.float16
AF = mybir.ActivationFunctionType
ALU = mybir.AluOpType

B_FULL = 16
N_CORES = 8
B_LOC = B_FULL // N_CORES  # 2
C = 128
H = W = 64
NPH = 32
NP = 1024
PS = 2
TOPK = 3
NCHUNK = NP // 128  # 8

N_DVE_MASK = 4   # q0/q1 chunks whose mask-mult runs on DVE (rest ACT+Pool)
N_DVE_TADD = 6   # F-phase T-adds on DVE (rest Pool)
N_DVE_S2 = 4     # q2 slab-adds on DVE (rest Pool)

LAST_EXEC_NS = None


def _free_bcast_ap(tile_ap, free_dims):
    ap = tile_ap
    new = [ap.ap[0]] + [list(d) for d in free_dims]
    return bass.AP(ap.tensor, ap.offset, new)


def build_nc():
    nc = bacc.Bacc(trn_type="TRN2")

    x_d = nc.dram_tensor("x", [B_LOC, C, H * W], F32, kind="ExternalInput")
    alpha_d = nc.dram_tensor("alpha", [128, 1], F32, kind="ExternalInput")
    mask_d = nc.dram_tensor("mask", [NP, NP], FP16, kind="ExternalInput")
    out_d = nc.dram_tensor("out", [B_LOC, 3, NP, TOPK], F32, kind="ExternalOutput")

    with TileContext(nc) as tc:
        with tc.tile_pool(name="const", bufs=1) as cpool, \
             tc.tile_pool(name="zp", bufs=1) as zpool, \
             tc.tile_pool(name="slab", bufs=2) as slabp, \
             tc.tile_pool(name="ssl", bufs=4) as sslp, \
             tc.tile_pool(name="work", bufs=3) as work, \
             tc.tile_pool(name="wsc", bufs=3) as wscp, \
             tc.tile_pool(name="small", bufs=3) as small, \
             tc.tile_pool(name="ps", bufs=2, space="PSUM") as ps, \
             tc.tile_pool(name="psn", bufs=2, space="PSUM") as psn, \
             tc.tile_pool(name="dsc", bufs=3, space="DRAM") as dsc:

            # ---- constants
            ones_k = cpool.tile([128, 1], FP16)   # colsum matmul lhsT
            nc.vector.memset(ones_k, 1.0)
            ones_r = cpool.tile([1, 128], FP16)   # K=1 bcast matmul lhsT
            nc.vector.memset(ones_r, 1.0)
            av = cpool.tile([128, 1], F32)        # alpha
            nc.sync.dma_start(av, alpha_d[:, :])
            av_h = cpool.tile([128, 1], F32)      # alpha/2
            nc.vector.tensor_scalar_mul(av_h, av, 0.5)
            av_d = cpool.tile([128, 1], F32)      # 2*alpha
            nc.vector.tensor_scalar_mul(av_d, av, 2.0)
            rav2 = cpool.tile([128, 1], F32)      # 1/(2*alpha)
            nc.vector.reciprocal(rav2, av_d)
            rav1 = cpool.tile([128, 1], F32)      # 1/alpha
            nc.vector.reciprocal(rav1, av)
            scale_E = [av, av, av_h]      # a_q for the rowsum exp
            scale_T = [av_d, av_d, av]    # 2 a_q for the tiny value exp
            scale_L = [rav2, rav2, rav1]  # 1/(2 a_q) for ln(u)

            # ---- mask (fp16, [p, i, m] chunk layout)
            mask_sb = cpool.tile([128, NCHUNK, NP], FP16)
            nc.sync.dma_start(
                mask_sb, mask_d[:, :].rearrange("(i p) m -> p i m", p=128))

            # ---- phase N
            chains = []
            for b in range(B_LOC):
                xs = slabp.tile([128, H * W], F32, name=f"xs{b}", tag="xs")
                nc.sync.dma_start(xs, x_d[b])
                xr = xs.rearrange("c (i r j s) -> c r s i j", r=PS, s=PS, j=NPH)
                for p in range(PS):
                    chains.append((b, p, xr[:, p, p]))

            inv = {}
            nrms = {}
            for b, p, zv in chains:  # nrm2 via DVE square + PE colsum
                zsq = work.tile([128, NP], FP16, name="zsq", tag="zsq", bufs=2)
                nc.vector.tensor_tensor(
                    out=zsq.rearrange("c (a b) -> c a b", a=NPH),
                    in0=zv, in1=zv, op=ALU.mult)
                nrm = psn.tile([1, NP], F32, name="nrm", tag="nrm", bufs=1)
                for h in range(2):
                    nc.tensor.matmul(nrm[:, 512 * h:512 * (h + 1)], ones_k,
                                     zsq[:, 512 * h:512 * (h + 1)],
                                     start=True, stop=True)
                nrms[(b, p)] = nrm
            lnns = {}
            for b, p, zv in chains:  # cluster the Lns, then the Exps
                lnn = small.tile([1, NP], F32, name="lnn", tag="lnn", bufs=2)
                nc.scalar.activation(lnn, nrms[(b, p)], AF.Ln)
                lnns[(b, p)] = lnn
            for b, p, zv in chains:  # inv = exp(-0.5 ln nrm2)
                inv1 = small.tile([1, NP], FP16, name="inv1", tag="inv1",
                                  bufs=2)
                nc.scalar.activation(inv1, lnns[(b, p)], AF.Exp, scale=-0.5)
                inv[(b, p)] = inv1
            zp = {}
            for b, p, zv in chains:
                ibc = psn.tile([128, NP], F32, name="ibc", tag="ibc", bufs=1)
                for h in range(2):
                    nc.tensor.matmul(ibc[:, 512 * h:512 * (h + 1)], ones_r,
                                     inv[(b, p)][:, 512 * h:512 * (h + 1)],
                                     start=True, stop=True)
                z = zpool.tile([128, NP], FP16, name=f"z{b}{p}", tag=f"z{b}{p}",
                               bufs=1)
                nc.vector.tensor_tensor(
                    out=z.rearrange("c (a b) -> c a b", a=NPH), in0=zv,
                    in1=ibc.rearrange("c (a b) -> c a b", a=NPH), op=ALU.mult)
                zp[(b, p)] = z

            # ---- phase M: 6 stages, per-chunk interleaved software pipeline
            s_of = {}  # (b, q) -> slab

            def E_chunk(b, q, i, s_sl, rT):
                if q < 2:
                    zs = zp[(b, q)]
                    G = ps.tile([128, NP], F32, name="G", tag="G")
                    for h in range(2):
                        nc.tensor.matmul(
                            G[:, 512 * h:512 * (h + 1)],
                            zs[:, 128 * i:128 * (i + 1)],
                            zs[:, 512 * h:512 * (h + 1)],
                            start=True, stop=True)
                    if i < N_DVE_MASK:
                        nc.vector.tensor_tensor(
                            out=s_sl[:, i, :], in0=G, in1=mask_sb[:, i, :],
                            op=ALU.mult)
                    else:
                        Gs = work.tile([128, NP], FP16, name="Gs", tag="Gs",
                                       bufs=3)
                        nc.scalar.copy(Gs, G)
                        nc.gpsimd.tensor_tensor(
                            out=s_sl[:, i, :], in0=Gs, in1=mask_sb[:, i, :],
                            op=ALU.mult)
                else:
                    s0, s1 = s_of[(b, 0)], s_of[(b, 1)]
                    if i % 4 == 0:  # batched 4-chunk add (DVE then Pool)
                        eng = nc.vector if i == 0 else nc.gpsimd
                        eng.tensor_tensor(
                            out=s_sl[:, i:i + 4, :], in0=s0[:, i:i + 4, :],
                            in1=s1[:, i:i + 4, :], op=ALU.add)
                e_scr = work.tile([128, NP], FP16, name="e_scr", tag="e_scr",
                                  bufs=2)
                nc.scalar.activation(e_scr, s_sl[:, i, :], AF.Exp,
                                     scale=scale_E[q],
                                     accum_out=rT[:, i:i + 1])

            def E_tail(b, q, s_sl, rT):
                u8 = small.tile([128, NCHUNK], F32, name="u8", tag="u8")
                nc.vector.reciprocal(u8, rT)
                lnu = small.tile([128, NCHUNK], F32, name="lnu", tag="lnu")
                nc.scalar.activation(lnu, u8, AF.Ln)
                lnsc = small.tile([128, NCHUNK], FP16, name="lnsc", tag="lnsc")
                nc.vector.tensor_scalar_mul(lnsc, lnu, scale_L[q][:, 0:1])
                l_dram = dsc.tile([NP], FP16, name="l_dram", tag="l_dram")
                nc.sync.dma_start(
                    l_dram[:].rearrange("(i p) -> p i", p=128), lnsc)
                lnubc = wscp.tile([128, NP], FP16, name="lnubc", tag="lnubc",
                                  bufs=2)
                src = bass.AP(l_dram[:].tensor, l_dram[:].offset,
                              [[0, 128], [1, NP]])
                nc.sync.dma_start(lnubc, src)
                return dict(s_sl=s_sl, lnubc=lnubc, u8=u8, b=b, q=q)

            def F_thalf(stg, h, t8):
                # batched T-add over 4 chunks: T = s + ln(u_m)/(2 a_q)
                T_bat = wscp.tile([128, 4, NP], FP16, name=f"T_bat{h}",
                                  tag=f"T_bat{h}", bufs=1)
                lnb = _free_bcast_ap(stg["lnubc"][:, :], [[0, 4], [1, NP]])
                nc.vector.tensor_tensor(
                    out=T_bat, in0=stg["s_sl"][:, 4 * h:4 * (h + 1), :],
                    in1=lnb, op=ALU.add)
                return T_bat

            def F_max8(stg, i, T_bat, t8):
                nc.vector.max(out=t8[:, i, :], in_=T_bat[:, i % 4, :])

            def F_tail(stg, t8):
                b, q = stg["b"], stg["q"]
                tex = small.tile([128, NCHUNK, TOPK], F32, name="tex",
                                 tag="tex")
                nc.scalar.activation(tex, t8[:, :, :TOPK], AF.Exp,
                                     scale=scale_T[q])
                oacc = small.tile([128, NCHUNK, TOPK], F32, name="oacc",
                                  tag="oacc")
                u8b = _free_bcast_ap(stg["u8"][:, :], [[1, NCHUNK], [0, TOPK]])
                nc.vector.tensor_tensor(out=oacc, in0=tex, in1=u8b,
                                        op=ALU.mult)
                dst = out_d[b, q].rearrange("(i p) k -> p i k", p=128)
                nc.sync.dma_start(dst, oacc)

            def emit_F_interleaved(pstg, pt8, echunk_fn):
                """Interleave F(prev) pieces between E(next) chunk emissions."""
                Tb = None
                for i in range(NCHUNK):
                    if echunk_fn is not None:
                        echunk_fn(i)
                    if i % 4 == 0:
                        Tb = F_thalf(pstg, i // 4, pt8)
                    F_max8(pstg, i, Tb, pt8)
                F_tail(pstg, pt8)

            stages = [(b, q) for b in range(B_LOC) for q in range(3)]
            pending = None   # (stg, t8) awaiting F emission
            for (b, q) in stages:
                s_sl = sslp.tile([128, NCHUNK, NP], FP16, name="s_sl",
                                 tag="s_sl")
                s_of[(b, q)] = s_sl
                rT = small.tile([128, NCHUNK], F32, name="rT", tag="rT")
                if pending is not None:
                    pstg, pt8 = pending
                    emit_F_interleaved(pstg, pt8,
                                       lambda i: E_chunk(b, q, i, s_sl, rT))
                else:
                    for i in range(NCHUNK):
                        E_chunk(b, q, i, s_sl, rT)
                stg = E_tail(b, q, s_sl, rT)
                t8 = small.tile([128, NCHUNK, 8], FP16, name="t8", tag="t8")
                pending = (stg, t8)
            pstg, pt8 = pending
            emit_F_interleaved(pstg, pt8, None)

    nc.compile()
    return nc


_NC_CACHE = None


def _get_nc():
    global _NC_CACHE
    if _NC_CACHE is None:
        _NC_CACHE = build_nc()
    return _NC_CACHE


def _build_mask() -> np.ndarray:
    rat_s = np.float32(0.05)
    sr = np.float32(NPH) * rat_s
    ind_r = np.arange(NPH, dtype=np.float32).reshape(1, NPH, 1)
    ind_c = np.arange(NPH, dtype=np.float32).reshape(1, 1, NPH)
    cent = np.arange(NPH, dtype=np.float32)
    cent_r = np.repeat(cent, NPH).reshape(NP, 1, 1)
    cent_c = np.tile(cent, NPH).reshape(NP, 1, 1)
    g = np.exp(-((ind_r - cent_r) ** 2) / (2.0 * sr * sr)) * np.exp(
        -((ind_c - cent_c) ** 2) / (2.0 * sr * sr)
    )
    return (1.0 - g).reshape(NP, NP).astype(np.float16)


def kernel(x: np.ndarray, alpha: np.ndarray) -> np.ndarray:
    global LAST_EXEC_NS
    x = np.ascontiguousarray(np.asarray(x, dtype=np.float32))
    alpha_arr = np.full((128, 1), np.float32(np.asarray(alpha)),
                        dtype=np.float32)
    mask = _build_mask()

    nc = _get_nc()
    in_maps = []
    for core in range(N_CORES):
        xs = x[core * B_LOC:(core + 1) * B_LOC].reshape(B_LOC, C, H * W)
        in_maps.append({"x": np.ascontiguousarray(xs), "alpha": alpha_arr,
                        "mask": mask})
    res = run_bass_kernel_spmd(nc, in_maps, core_ids=list(range(N_CORES)))
    LAST_EXEC_NS = res.exec_time_ns

    out = np.empty((B_FULL, TOPK, H, W), dtype=np.float32)
    for core in range(N_CORES):
        t = res.results[core]["out"]
        for bl in range(B_LOC):
            bg = core * B_LOC + bl
            tq = t[bl].reshape(3, NPH, NPH, TOPK).transpose(0, 3, 1, 2)
            out[bg, :, 0::2, 0::2] = tq[0]
            out[bg, :, 1::2, 1::2] = tq[1]
            out[bg, :, 0::2, 1::2] = tq[2]
            out[bg, :, 1::2, 0::2] = tq[2]
    return out
